# revision 20
# baseline (speedup 1.0000x reference)
"""NF4-quantized LoRA linear layer on 8 Trainium2 NeuronCores.

Computation (reference):
    w = NF4_TABLE[w_codes] * w_scales[block-expanded]        # [O, I]
    out = x @ w.T + (alpha/rank) * (x @ lora_a.T) @ lora_b.T # [B, S, O]

Strategy:
  - Tensor-parallel split of the output dim across 8 cores (O_SH = 512 each).
    Every core sees all of x; no collectives; host concatenates outputs.
  - The LoRA path is folded into the weights once per core:
    W_eff = dequant(codes) * scales + (alpha/rank) * lora_a.T @ lora_b.T,
    so the steady-state loop is a single dense bf16 matmul.
  - NF4 dequant runs on Vector + Scalar engines as an exact hybrid
    step/ramp chain in fp16 (max abs table err ~3e-4), on [128, 2048]
    macro-tiles (4 i-tiles fused) to amortize DVE overheads.
  - The contraction dim is split into 2 phases (10/22 i-tiles) with a DRAM
    partial, so the PE starts matmuls as soon as the first few weight
    macro-tiles are dequantized instead of waiting for all of them.
"""

import numpy as np
import ml_dtypes

import concourse.mybir as mybir
import concourse.tile as tile
from concourse import bacc
from concourse.bass_utils import run_bass_kernel_spmd

NF4_TABLE = np.array(
    [
        -1.0, -0.6961928009986877, -0.5250730514526367, -0.39491748809814453,
        -0.28444138169288635, -0.18477343022823334, -0.09105003625154495, 0.0,
        0.07958029955625534, 0.16093020141124725, 0.24611230194568634,
        0.33791524171829224, 0.44070982933044434, 0.5626170039176941,
        0.7229568362236328, 1.0,
    ],
    dtype=np.float64,
)

B, S, I, O, R, BLK = 4, 2048, 4096, 4096, 16, 64
M = B * S                      # 8192 token rows
N_CORES = 8
O_SH = O // N_CORES            # 512 output cols per core
IT = I // 128                  # 32 contraction tiles
MT = M // 128                  # 64 row tiles
# dequant macro-tile i-ranges: a half-size macro at it 8-10 aligns weight
# availability with the phase-A boundary so the PE gate is ~3 macros early
MACRO_RANGES = [(0, 4), (4, 8), (8, 10), (10, 14), (14, 18), (18, 22),
                (22, 26), (26, 30), (30, 32)]
PHASES = [(0, 10), (10, 32)]   # i-tile ranges per m-loop phase
LORA_SCALE = 2.0               # alpha / rank

# Hybrid exact NF4 chain: t[c] = -1 + sum_S d_v*[c>=v] + sum_R g_v*relu(c-v)
# DVE computes the steps (tensor_scalar is_ge*delta + add chain), ACT the
# relu ramps; constants solved + fp16-greedy-tuned offline (max err 3.2e-4).
S_KNOTS = [1, 2, 3, 4, 6, 8, 10, 12, 14, 15]
DELTAS = [0.3037109375, 0.171142578125, 0.1302490234375, 0.1104736328125,
          -0.00594329833984375, -0.01146697998046875, 0.0038318634033203125,
          0.01099395751953125, 0.038421630859375, 0.1551513671875]
R_KNOTS = [4, 6, 8, 10, 12]
GAMMAS = [0.09966795146465302, -0.008617915213108063, -0.00970013439655304,
          0.010453037917613983, 0.03010423481464386]

F16 = mybir.dt.float16
BF16 = mybir.dt.bfloat16
F32 = mybir.dt.float32
ALU = mybir.AluOpType
ACTF = mybir.ActivationFunctionType

BF16_NP = ml_dtypes.bfloat16


def _build_nc():
    nc = bacc.Bacc("TRN2", target_bir_lowering=False, debug=False,
                   num_devices=N_CORES)

    xt = nc.dram_tensor("xt", [128, MT, IT, 128], BF16, kind="ExternalInput")
    codes = nc.dram_tensor("codes", [I, O_SH], F16, kind="ExternalInput")
    scales = nc.dram_tensor("scales", [I, O_SH], F16, kind="ExternalInput")
    la = nc.dram_tensor("la", [R, I], BF16, kind="ExternalInput")
    lb = nc.dram_tensor("lb", [R, O_SH], BF16, kind="ExternalInput")
    out = nc.dram_tensor("out", [M, O_SH], F32, kind="ExternalOutput")

    codes_r = codes.ap().rearrange("(t p) o -> p t o", p=128)
    scales_r = scales.ap().rearrange("(t p) o -> p t o", p=128)

    with tile.TileContext(nc) as tc:
        with (
            tc.tile_pool(name="wpool", bufs=len(MACRO_RANGES)) as wpool,
            tc.tile_pool(name="wlpool", bufs=len(MACRO_RANGES)) as wlpool,
            tc.tile_pool(name="dq", bufs=3) as dq,
            tc.tile_pool(name="xpool", bufs=4) as xpool,
            tc.tile_pool(name="cpool", bufs=1) as cpool,
            tc.tile_pool(name="opool", bufs=3) as opool,
            tc.tile_pool(name="dram", bufs=1, space="DRAM") as dram,
            tc.tile_pool(name="ps_a", bufs=3, space="PSUM") as pp_a,
            tc.tile_pool(name="ps_b", bufs=3, space="PSUM") as pp_b,
            tc.tile_pool(name="ps_l", bufs=2, space="PSUM") as pp_l,
        ):
            pp_phase = [pp_a, pp_b]
            part = dram.tile([M, O_SH], F32, tag="part")

            # constants
            la_sb = cpool.tile([R, I], BF16, tag="la")
            nc.sync.dma_start(la_sb[:], la.ap())
            lb_sb = cpool.tile([R, O_SH], BF16, tag="lb")
            nc.sync.dma_start(lb_sb[:], lb.ap())
            neg1 = cpool.tile([128, 4 * O_SH], F16, tag="neg1")
            nc.vector.memset(neg1[:], -1.0)
            biases = []
            for v, g in zip(R_KNOTS, GAMMAS):
                bt = cpool.tile([128, 1], F32, tag=f"bias{v}")
                nc.vector.memset(bt[:], -abs(g) * v)
                biases.append(bt)

            # ---- LoRA weight fold: W_lora[i, o] = la.T @ lb, per i-tile ----
            wl_tiles = []
            for it_lo, it_hi in MACRO_RANGES:
                nt = it_hi - it_lo
                wl = wlpool.tile([128, nt * O_SH], F16, tag="wl")
                for j in range(nt):
                    it = it_lo + j
                    pl = pp_l.tile([128, O_SH], F32, tag="pl")
                    nc.tensor.matmul(
                        pl[:], la_sb[:, it * 128:(it + 1) * 128], lb_sb[:],
                        start=True, stop=True,
                    )
                    nc.scalar.copy(wl[:, j * O_SH:(j + 1) * O_SH], pl[:])
                wl_tiles.append(wl)

            # ---- NF4 dequant on [128, nt*512] macro-tiles ----
            w_aps = {}   # global i-tile -> AP into its macro tile
            for mi, (it_lo, it_hi) in enumerate(MACRO_RANGES):
                nt = it_hi - it_lo
                fd = nt * O_SH
                ct = dq.tile([128, fd], F16, tag="ct")
                st = dq.tile([128, fd], F16, tag="st")
                nc.sync.dma_start(
                    ct[:].rearrange("p (t o) -> p t o", t=nt),
                    codes_r[:, it_lo:it_hi, :],
                )
                nc.sync.dma_start(
                    st[:].rearrange("p (t o) -> p t o", t=nt),
                    scales_r[:, it_lo:it_hi, :],
                )
                acc = dq.tile([128, fd], F16, tag="acc")
                nc.vector.tensor_scalar(
                    acc[:], ct[:], S_KNOTS[0] - 0.5, DELTAS[0],
                    op0=ALU.is_ge, op1=ALU.mult,
                )
                nc.vector.tensor_tensor(
                    acc[:], acc[:], neg1[:, :fd], op=ALU.add
                )
                for v, dv in zip(S_KNOTS[1:], DELTAS[1:]):
                    mv = dq.tile([128, fd], F16, tag="mv")
                    nc.vector.tensor_scalar(
                        mv[:], ct[:], v - 0.5, dv,
                        op0=ALU.is_ge, op1=ALU.mult,
                    )
                    nc.vector.tensor_tensor(acc[:], acc[:], mv[:], op=ALU.add)
                for (v, g), bt in zip(zip(R_KNOTS, GAMMAS), biases):
                    ramp = dq.tile([128, fd], F16, tag="ramp")
                    nc.scalar.activation(
                        ramp[:], ct[:], ACTF.Relu, bias=bt[:], scale=abs(g)
                    )
                    nc.vector.tensor_tensor(
                        acc[:], acc[:], ramp[:],
                        op=ALU.add if g > 0 else ALU.subtract,
                    )
                # w = acc * scale + W_lora  (fp32 internal, bf16 store)
                wtmp = dq.tile([128, fd], F16, tag="wtmp")
                nc.vector.tensor_tensor(wtmp[:], acc[:], st[:], op=ALU.mult)
                wt = wpool.tile([128, fd], BF16, tag="w")
                nc.vector.tensor_tensor(
                    wt[:], wtmp[:], wl_tiles[mi][:], op=ALU.add
                )
                for j, it in enumerate(range(it_lo, it_hi)):
                    w_aps[it] = wt[:, j * O_SH:(j + 1) * O_SH]

            def w_ap(it):
                return w_aps[it]

            # ---- m-loop in 3 phases over i ----
            for ph, (i_lo, i_hi) in enumerate(PHASES):
                n_it = i_hi - i_lo
                for mt in range(MT):
                    xa = xpool.tile([128, n_it, 128], BF16, tag=f"x{ph}")
                    nc.sync.dma_start(xa[:], xt.ap()[:, mt, i_lo:i_hi, :])
                    po = pp_phase[ph].tile([128, O_SH], F32, tag=f"p{ph}")
                    for k, it in enumerate(range(i_lo, i_hi)):
                        nc.tensor.matmul(
                            po[:], xa[:, k, :], w_ap(it),
                            start=(k == 0), stop=(k == n_it - 1),
                        )
                    mrow = part[mt * 128:(mt + 1) * 128, :]
                    if ph == 0:
                        ev = opool.tile([128, O_SH], F32, tag=f"ev{ph}")
                        nc.scalar.copy(ev[:], po[:])
                        nc.sync.dma_start(mrow, ev[:])
                    else:
                        psb = opool.tile([128, O_SH], F32, tag="psb")
                        nc.sync.dma_start(psb[:], mrow)
                        ev = opool.tile([128, O_SH], F32, tag=f"ev{ph}")
                        nc.vector.tensor_tensor(
                            ev[:], po[:], psb[:], op=ALU.add
                        )
                        nc.sync.dma_start(
                            out.ap()[mt * 128:(mt + 1) * 128, :], ev[:]
                        )

    nc.compile()
    return nc


_NC_CACHE = {}


def _get_nc():
    if "nc" not in _NC_CACHE:
        _NC_CACHE["nc"] = _build_nc()
    return _NC_CACHE["nc"]


def prepare_in_maps(x, w_codes, w_scales, lora_a, lora_b):
    """Host-side sharding + layout prep (no arithmetic beyond casts/folds)."""
    xm = np.ascontiguousarray(x.reshape(M, I))
    # xt[p, mt, t, mm] = x[mt*128+mm, t*128+p], bf16
    xtl = (
        xm.T.reshape(IT, 128, MT, 128)
        .transpose(1, 2, 0, 3)
        .astype(BF16_NP)
    )
    xtl = np.ascontiguousarray(xtl)

    la = np.ascontiguousarray(
        (LORA_SCALE * lora_a.astype(np.float64)).astype(BF16_NP)
    )

    in_maps = []
    for c in range(N_CORES):
        o_lo, o_hi = c * O_SH, (c + 1) * O_SH
        codes_t = np.ascontiguousarray(
            w_codes[o_lo:o_hi].T.astype(np.float16)
        )
        scales_t = np.ascontiguousarray(
            np.repeat(w_scales[o_lo:o_hi].T, BLK, axis=0).astype(np.float16)
        )
        lb_t = np.ascontiguousarray(lora_b[o_lo:o_hi].T.astype(BF16_NP))
        in_maps.append(
            {
                "xt": xtl,
                "codes": codes_t,
                "scales": scales_t,
                "la": la,
                "lb": lb_t,
            }
        )
    return in_maps


def run(in_maps, trace=False, retries=2):
    nc = _get_nc()
    last = None
    for attempt in range(retries + 1):
        try:
            return run_bass_kernel_spmd(
                nc, in_maps, core_ids=list(range(N_CORES)), trace=trace
            )
        except Exception as e:  # transient NRT/axon device errors
            last = e
            if attempt == retries:
                raise
            import time as _time

            _time.sleep(5)
    raise last


def kernel(x, w_codes, w_scales, lora_a, lora_b):
    in_maps = prepare_in_maps(x, w_codes, w_scales, lora_a, lora_b)
    res = run(in_maps, trace=False)
    out = np.concatenate(
        [res.results[c]["out"] for c in range(N_CORES)], axis=1
    )
    return out.reshape(B, S, O).astype(np.float32)


# revision 21
# speedup vs baseline: 1.0026x; 1.0026x over previous
"""NF4-quantized LoRA linear layer on 8 Trainium2 NeuronCores.

Computation (reference):
    w = NF4_TABLE[w_codes] * w_scales[block-expanded]        # [O, I]
    out = x @ w.T + (alpha/rank) * (x @ lora_a.T) @ lora_b.T # [B, S, O]

Strategy:
  - Tensor-parallel split of the output dim across 8 cores (O_SH = 512 each).
    Every core sees all of x; no collectives; host concatenates outputs.
  - The LoRA path is folded into the weights once per core:
    W_eff = dequant(codes) * scales + (alpha/rank) * lora_a.T @ lora_b.T,
    so the steady-state loop is a single dense bf16 matmul.
  - NF4 dequant runs on Vector + Scalar engines as an exact hybrid
    step/ramp chain in fp16 (max abs table err ~3e-4), on [128, 2048]
    macro-tiles (4 i-tiles fused) to amortize DVE overheads.
  - The contraction dim is split into 2 phases (10/22 i-tiles) with a DRAM
    partial, so the PE starts matmuls as soon as the first few weight
    macro-tiles are dequantized instead of waiting for all of them.
"""

import numpy as np
import ml_dtypes

import concourse.mybir as mybir
import concourse.tile as tile
from concourse import bacc
from concourse.bass_utils import run_bass_kernel_spmd

NF4_TABLE = np.array(
    [
        -1.0, -0.6961928009986877, -0.5250730514526367, -0.39491748809814453,
        -0.28444138169288635, -0.18477343022823334, -0.09105003625154495, 0.0,
        0.07958029955625534, 0.16093020141124725, 0.24611230194568634,
        0.33791524171829224, 0.44070982933044434, 0.5626170039176941,
        0.7229568362236328, 1.0,
    ],
    dtype=np.float64,
)

B, S, I, O, R, BLK = 4, 2048, 4096, 4096, 16, 64
M = B * S                      # 8192 token rows
N_CORES = 8
O_SH = O // N_CORES            # 512 output cols per core
IT = I // 128                  # 32 contraction tiles
MT = M // 128                  # 64 row tiles
# dequant macro-tile i-ranges: a half-size macro at it 8-10 aligns weight
# availability with the phase-A boundary so the PE gate is ~3 macros early
MACRO_RANGES = [(0, 4), (4, 8), (8, 10), (10, 14), (14, 18), (18, 22),
                (22, 26), (26, 30), (30, 32)]
PHASES = [(0, 10), (10, 32)]   # i-tile ranges per m-loop phase
LORA_SCALE = 2.0               # alpha / rank

# Hybrid exact NF4 chain: t[c] = -1 + sum_S d_v*[c>=v] + sum_R g_v*relu(c-v)
# DVE computes the steps (tensor_scalar is_ge*delta + add chain), ACT the
# relu ramps; constants solved + fp16-greedy-tuned offline (max err 3.2e-4).
S_KNOTS = [1, 2, 3, 4, 6, 8, 10, 12, 14, 15]
DELTAS = [0.3037109375, 0.171142578125, 0.1302490234375, 0.1104736328125,
          -0.00594329833984375, -0.01146697998046875, 0.0038318634033203125,
          0.01099395751953125, 0.038421630859375, 0.1551513671875]
R_KNOTS = [4, 6, 8, 10, 12]
GAMMAS = [0.09966795146465302, -0.008617915213108063, -0.00970013439655304,
          0.010453037917613983, 0.03010423481464386]

F16 = mybir.dt.float16
BF16 = mybir.dt.bfloat16
F32 = mybir.dt.float32
ALU = mybir.AluOpType
ACTF = mybir.ActivationFunctionType

BF16_NP = ml_dtypes.bfloat16


def _build_nc():
    nc = bacc.Bacc("TRN2", target_bir_lowering=False, debug=False,
                   num_devices=N_CORES)

    xt = nc.dram_tensor("xt", [128, MT, IT, 128], BF16, kind="ExternalInput")
    codes = nc.dram_tensor("codes", [I, O_SH], F16, kind="ExternalInput")
    scales = nc.dram_tensor("scales", [I, O_SH], F16, kind="ExternalInput")
    la = nc.dram_tensor("la", [R, I], BF16, kind="ExternalInput")
    lb = nc.dram_tensor("lb", [R, O_SH], BF16, kind="ExternalInput")
    out = nc.dram_tensor("out", [M, O_SH], F32, kind="ExternalOutput")

    codes_r = codes.ap().rearrange("(t p) o -> p t o", p=128)
    scales_r = scales.ap().rearrange("(t p) o -> p t o", p=128)

    with tile.TileContext(nc) as tc:
        with (
            tc.tile_pool(name="wpool", bufs=len(MACRO_RANGES)) as wpool,
            tc.tile_pool(name="wlpool", bufs=len(MACRO_RANGES)) as wlpool,
            tc.tile_pool(name="dq", bufs=3) as dq,
            tc.tile_pool(name="xpool", bufs=4) as xpool,
            tc.tile_pool(name="cpool", bufs=1) as cpool,
            tc.tile_pool(name="opool", bufs=3) as opool,
            tc.tile_pool(name="dram", bufs=1, space="DRAM") as dram,
            tc.tile_pool(name="ps_a", bufs=4, space="PSUM") as pp_a,
            tc.tile_pool(name="ps_b", bufs=3, space="PSUM") as pp_b,
            tc.tile_pool(name="ps_l", bufs=1, space="PSUM") as pp_l,
        ):
            pp_phase = [pp_a, pp_b]
            part = dram.tile([M, O_SH], F32, tag="part")

            # constants
            la_sb = cpool.tile([R, I], BF16, tag="la")
            nc.sync.dma_start(la_sb[:], la.ap())
            lb_sb = cpool.tile([R, O_SH], BF16, tag="lb")
            nc.sync.dma_start(lb_sb[:], lb.ap())
            neg1 = cpool.tile([128, 4 * O_SH], F16, tag="neg1")
            nc.vector.memset(neg1[:], -1.0)
            biases = []
            for v, g in zip(R_KNOTS, GAMMAS):
                bt = cpool.tile([128, 1], F32, tag=f"bias{v}")
                nc.vector.memset(bt[:], -abs(g) * v)
                biases.append(bt)

            # ---- LoRA weight fold: W_lora[i, o] = la.T @ lb, per i-tile ----
            wl_tiles = []
            for it_lo, it_hi in MACRO_RANGES:
                nt = it_hi - it_lo
                wl = wlpool.tile([128, nt * O_SH], F16, tag="wl")
                for j in range(nt):
                    it = it_lo + j
                    pl = pp_l.tile([128, O_SH], F32, tag="pl")
                    nc.tensor.matmul(
                        pl[:], la_sb[:, it * 128:(it + 1) * 128], lb_sb[:],
                        start=True, stop=True,
                    )
                    nc.scalar.copy(wl[:, j * O_SH:(j + 1) * O_SH], pl[:])
                wl_tiles.append(wl)

            # ---- NF4 dequant on [128, nt*512] macro-tiles ----
            w_aps = {}   # global i-tile -> AP into its macro tile
            for mi, (it_lo, it_hi) in enumerate(MACRO_RANGES):
                nt = it_hi - it_lo
                fd = nt * O_SH
                ct = dq.tile([128, fd], F16, tag="ct")
                st = dq.tile([128, fd], F16, tag="st")
                nc.sync.dma_start(
                    ct[:].rearrange("p (t o) -> p t o", t=nt),
                    codes_r[:, it_lo:it_hi, :],
                )
                nc.sync.dma_start(
                    st[:].rearrange("p (t o) -> p t o", t=nt),
                    scales_r[:, it_lo:it_hi, :],
                )
                acc = dq.tile([128, fd], F16, tag="acc")
                nc.vector.tensor_scalar(
                    acc[:], ct[:], S_KNOTS[0] - 0.5, DELTAS[0],
                    op0=ALU.is_ge, op1=ALU.mult,
                )
                nc.vector.tensor_tensor(
                    acc[:], acc[:], neg1[:, :fd], op=ALU.add
                )
                for v, dv in zip(S_KNOTS[1:], DELTAS[1:]):
                    mv = dq.tile([128, fd], F16, tag="mv")
                    nc.vector.tensor_scalar(
                        mv[:], ct[:], v - 0.5, dv,
                        op0=ALU.is_ge, op1=ALU.mult,
                    )
                    nc.vector.tensor_tensor(acc[:], acc[:], mv[:], op=ALU.add)
                for (v, g), bt in zip(zip(R_KNOTS, GAMMAS), biases):
                    ramp = dq.tile([128, fd], F16, tag="ramp")
                    nc.scalar.activation(
                        ramp[:], ct[:], ACTF.Relu, bias=bt[:], scale=abs(g)
                    )
                    nc.vector.tensor_tensor(
                        acc[:], acc[:], ramp[:],
                        op=ALU.add if g > 0 else ALU.subtract,
                    )
                # w = acc * scale + W_lora  (fp32 internal, bf16 store)
                wtmp = dq.tile([128, fd], F16, tag="wtmp")
                nc.vector.tensor_tensor(wtmp[:], acc[:], st[:], op=ALU.mult)
                wt = wpool.tile([128, fd], BF16, tag="w")
                nc.vector.tensor_tensor(
                    wt[:], wtmp[:], wl_tiles[mi][:], op=ALU.add
                )
                for j, it in enumerate(range(it_lo, it_hi)):
                    w_aps[it] = wt[:, j * O_SH:(j + 1) * O_SH]

            def w_ap(it):
                return w_aps[it]

            # ---- m-loop in 3 phases over i ----
            for ph, (i_lo, i_hi) in enumerate(PHASES):
                n_it = i_hi - i_lo
                for mt in range(MT):
                    xa = xpool.tile([128, n_it, 128], BF16, tag=f"x{ph}")
                    nc.sync.dma_start(xa[:], xt.ap()[:, mt, i_lo:i_hi, :])
                    po = pp_phase[ph].tile([128, O_SH], F32, tag=f"p{ph}")
                    for k, it in enumerate(range(i_lo, i_hi)):
                        nc.tensor.matmul(
                            po[:], xa[:, k, :], w_ap(it),
                            start=(k == 0), stop=(k == n_it - 1),
                        )
                    mrow = part[mt * 128:(mt + 1) * 128, :]
                    if ph == 0:
                        ev = opool.tile([128, O_SH], F32, tag=f"ev{ph}")
                        nc.scalar.copy(ev[:], po[:])
                        nc.sync.dma_start(mrow, ev[:])
                    else:
                        psb = opool.tile([128, O_SH], F32, tag="psb")
                        nc.sync.dma_start(psb[:], mrow)
                        ev = opool.tile([128, O_SH], F32, tag=f"ev{ph}")
                        nc.vector.tensor_tensor(
                            ev[:], po[:], psb[:], op=ALU.add
                        )
                        nc.sync.dma_start(
                            out.ap()[mt * 128:(mt + 1) * 128, :], ev[:]
                        )

    nc.compile()
    return nc


_NC_CACHE = {}


def _get_nc():
    if "nc" not in _NC_CACHE:
        _NC_CACHE["nc"] = _build_nc()
    return _NC_CACHE["nc"]


def prepare_in_maps(x, w_codes, w_scales, lora_a, lora_b):
    """Host-side sharding + layout prep (no arithmetic beyond casts/folds)."""
    xm = np.ascontiguousarray(x.reshape(M, I))
    # xt[p, mt, t, mm] = x[mt*128+mm, t*128+p], bf16
    xtl = (
        xm.T.reshape(IT, 128, MT, 128)
        .transpose(1, 2, 0, 3)
        .astype(BF16_NP)
    )
    xtl = np.ascontiguousarray(xtl)

    la = np.ascontiguousarray(
        (LORA_SCALE * lora_a.astype(np.float64)).astype(BF16_NP)
    )

    in_maps = []
    for c in range(N_CORES):
        o_lo, o_hi = c * O_SH, (c + 1) * O_SH
        codes_t = np.ascontiguousarray(
            w_codes[o_lo:o_hi].T.astype(np.float16)
        )
        scales_t = np.ascontiguousarray(
            np.repeat(w_scales[o_lo:o_hi].T, BLK, axis=0).astype(np.float16)
        )
        lb_t = np.ascontiguousarray(lora_b[o_lo:o_hi].T.astype(BF16_NP))
        in_maps.append(
            {
                "xt": xtl,
                "codes": codes_t,
                "scales": scales_t,
                "la": la,
                "lb": lb_t,
            }
        )
    return in_maps


def run(in_maps, trace=False, retries=2):
    nc = _get_nc()
    last = None
    for attempt in range(retries + 1):
        try:
            return run_bass_kernel_spmd(
                nc, in_maps, core_ids=list(range(N_CORES)), trace=trace
            )
        except Exception as e:  # transient NRT/axon device errors
            last = e
            if attempt == retries:
                raise
            import time as _time

            _time.sleep(5)
    raise last


def kernel(x, w_codes, w_scales, lora_a, lora_b):
    in_maps = prepare_in_maps(x, w_codes, w_scales, lora_a, lora_b)
    res = run(in_maps, trace=False)
    out = np.concatenate(
        [res.results[c]["out"] for c in range(N_CORES)], axis=1
    )
    return out.reshape(B, S, O).astype(np.float32)
